# revision 1
# baseline (speedup 1.0000x reference)
"""Trainium2 Bass kernel: y = x @ weight.T + bias  (4096x4096x4096, fp32).

Sharding: data-parallel over the batch dim — each of the 8 NeuronCores
computes y[c*512:(c+1)*512] = x[c*512:(c+1)*512] @ W.T + bias with the
full weight replicated.

Per-core algorithm (all on device):
  - The tensor engine contracts over the partition dim, so both operands
    need K on partitions; x and W are stored K-contiguous.  fp32 has no
    DMA transpose, so both are transposed on the fly with PE-transpose
    (matmul transpose mode against an identity) + PSUM->SBUF eviction.
  - Matmuls run in float32r (rounded fp32, ~12-bit mantissa): 1 cyc/row
    at free dim >= 256 vs 4 cyc/row for plain fp32.
  - bias is folded into the PSUM accumulation with a K=1 ones-row matmul
    (start=True), avoiding a partition-broadcast on the vector engine.

Loop structure per core:
  Phase A: build xT [128, KT, B_S] in SBUF (lhsT tiles [128k, 128b])
  Phase B: for og (8 o-groups of 512):
      bias matmul into 4 psum banks (one per 128-row b-tile)
      for kc (4 k-chunks of 1024):
          DMA W chunk [128, 4ob, 1024] (natural layout, 4KB runs)
          for kt in chunk:  # 8
              4x PE-transpose -> wT_ps [128, 512]; evict -> wT (f32r)
              4x matmul(psum_y[bt], xT[:, k, bt*128:], wT)
      evict psum_y (+DMA out y rows)
"""
import numpy as np

import concourse.bass as bass
import concourse.mybir as mybir
import concourse.tile as tile
from concourse import bacc
from concourse.masks import make_identity
from concourse.bass_utils import run_bass_kernel_spmd

F32 = mybir.dt.float32
F32R = mybir.dt.float32r
P = 128

N_CORES = 8
B = 4096
K = 4096
O = 4096
B_S = B // N_CORES   # 512 batch rows per core


def build(B_S=B_S, K=K, O=O, OG=512, KC=8, n_cores=N_CORES):
    """OG: o-group width (psum free dim). KC: k-tiles per W dma chunk."""
    KT = K // P           # 32 k tiles
    BT = B_S // P         # 4 b tiles (psum banks for y)
    NOG = O // OG         # o-groups
    OB = OG // P          # 128-blocks per o-group
    NKC = KT // KC        # w-dma chunks per o-group

    nc = bacc.Bacc("TRN2", target_bir_lowering=False, debug=False,
                   num_devices=n_cores)
    x = nc.dram_tensor("x", [B_S, K], F32R, kind="ExternalInput").ap()
    w = nc.dram_tensor("w", [O, K], F32R, kind="ExternalInput").ap()
    b = nc.dram_tensor("b", [O], F32, kind="ExternalInput").ap()
    y = nc.dram_tensor("y", [B_S, O], F32, kind="ExternalOutput").ap()

    with tile.TileContext(nc) as tc:
        with tc.tile_pool(name="const", bufs=1) as const, \
             tc.tile_pool(name="xna", bufs=4) as xna_pool, \
             tc.tile_pool(name="xt", bufs=1) as xt_pool, \
             tc.tile_pool(name="wna", bufs=2) as wna_pool, \
             tc.tile_pool(name="wt", bufs=3) as wt_pool, \
             tc.tile_pool(name="yo", bufs=2) as yo_pool, \
             tc.tile_pool(name="tps", bufs=3, space="PSUM") as tps, \
             tc.tile_pool(name="yps", bufs=1, space="PSUM") as yps:

            ident_f = const.tile([P, P], F32)
            make_identity(nc, ident_f)
            ident = const.tile([P, P], F32R)
            nc.vector.tensor_copy(ident, ident_f)

            bias_sb = const.tile([1, O], F32R)
            nc.sync.dma_start(bias_sb, b.unsqueeze(0).bitcast(F32R))
            ones_f = const.tile([1, P], F32)
            nc.any.memset(ones_f, 1.0)
            ones_k1 = const.tile([1, P], F32R)
            nc.vector.tensor_copy(ones_k1, ones_f)

            # ---- Phase A: x -> xT ----
            xT = xt_pool.tile([P, KT, B_S], F32R)  # [k, kt, b]
            for bt in range(BT):
                x_nat = xna_pool.tile([P, K], F32R, tag="x_nat")
                nc.sync.dma_start(x_nat, x[bt * P:(bt + 1) * P, :])
                for kt in range(KT):
                    xt_ps = tps.tile([P, P], F32R, tag="t_ps")
                    nc.tensor.transpose(
                        xt_ps, x_nat[:, kt * P:(kt + 1) * P], ident)
                    nc.any.tensor_copy(
                        xT[:, kt, bt * P:(bt + 1) * P], xt_ps)

            # ---- Phase B ----
            for og in range(NOG):
                psum_y = [yps.tile([P, OG], F32, name=f"psum_y{og}_{bt}",
                                   tag=f"psum_y{bt}")
                          for bt in range(BT)]
                for bt in range(BT):
                    nc.tensor.matmul(
                        psum_y[bt], ones_k1,
                        bias_sb[:, og * OG:(og + 1) * OG],
                        start=True, stop=False)
                for kc in range(NKC):
                    w_nat = wna_pool.tile([P, OB, KC * P], F32R, tag="w_nat")
                    nc.sync.dma_start(
                        w_nat,
                        w[og * OG:(og + 1) * OG, kc * KC * P:(kc + 1) * KC * P]
                        .rearrange("(ob p) k -> p ob k", p=P))
                    for kt in range(KC):
                        k = kc * KC + kt
                        wt_ps = tps.tile([P, OG], F32R, tag="t_ps")
                        for ob in range(OB):
                            nc.tensor.transpose(
                                wt_ps[:, ob * P:(ob + 1) * P],
                                w_nat[:, ob, kt * P:(kt + 1) * P],
                                ident)
                        wT = wt_pool.tile([P, OG], F32R, tag="wT")
                        nc.any.tensor_copy(wT, wt_ps)
                        for bt in range(BT):
                            nc.tensor.matmul(
                                psum_y[bt],
                                xT[:, k, bt * P:(bt + 1) * P],
                                wT,
                                start=False,
                                stop=(k == KT - 1),
                            )
                for bt in range(BT):
                    y_sb = yo_pool.tile([P, OG], F32, tag="y_sb")
                    nc.any.tensor_copy(y_sb, psum_y[bt])
                    nc.sync.dma_start(
                        y[bt * P:(bt + 1) * P, og * OG:(og + 1) * OG], y_sb)

    nc.compile()
    return nc


_nc_cache = {}


def get_nc():
    if "nc" not in _nc_cache:
        _nc_cache["nc"] = build()
    return _nc_cache["nc"]


def make_in_maps(x, weight, bias):
    x = np.ascontiguousarray(np.asarray(x, dtype=np.float32))
    weight = np.ascontiguousarray(np.asarray(weight, dtype=np.float32))
    bias = np.ascontiguousarray(np.asarray(bias, dtype=np.float32))
    assert x.shape == (B, K) and weight.shape == (O, K) and bias.shape == (O,)
    return [
        {"x": x[c * B_S:(c + 1) * B_S], "w": weight, "b": bias}
        for c in range(N_CORES)
    ]


def run(x, weight, bias, **spmd_kwargs):
    """Run on all 8 cores; returns (y_full, BassKernelResults)."""
    nc = get_nc()
    in_maps = make_in_maps(x, weight, bias)
    res = run_bass_kernel_spmd(nc, in_maps, list(range(N_CORES)), **spmd_kwargs)
    y = np.concatenate([res.results[c]["y"] for c in range(N_CORES)], axis=0)
    return y.astype(np.float32, copy=False), res


def kernel(x, weight, bias):
    y, _ = run(x, weight, bias)
    return y



# revision 6
# speedup vs baseline: 1.0104x; 1.0104x over previous
"""Trainium2 Bass kernel: y = x @ weight.T + bias  (4096x4096x4096, fp32 in/out).

Sharding: 2-D (4 batch x 2 out) over the 8 NeuronCores. Core c = (bi, oi)
with bi = c // 2, oi = c % 2 computes
  y[bi*1024:(bi+1)*1024, oi*2048:(oi+1)*2048]
from x_s = x[bi*1024:+1024] (full K) and W_s = weight[oi*2048:+2048] (full K).
This halves per-core W traffic vs pure data-parallel (58.8 MB vs 84 MB HBM).

Per-core algorithm (all on device):
  - Matmuls run in bf16 (1 cyc/row on the PE, same rate as fp32r; psum
    accumulation stays fp32). Inputs are cast fp32->bf16 on the
    scalar/vector engines.
  - Both operands need K on partitions; bf16 enables the XBAR DMA
    transpose (16x128 tiles), so NO PE-transposes: the tensor engine does
    only the 1024 y-matmuls (free dim 512) + 32 K=1 bias matmuls.
    One XBAR call transposes a whole [128, 4096] row-block into
    out[p, kt, r] = in[r, kt*128+p] (verified 3-D output semantics).
  - bias is folded into PSUM with a K=1 ones-row matmul (start=True).
  - All 8 psum banks hold one o-group (512 wide), one bank per b-tile.
    Loop order (og, bt, k): a b-tile's 32 matmuls only need its own xT
    slice, and its eviction overlaps the next b-tile's matmuls, so
    neither pipeline fill nor o-group boundaries stall the PE.

Engine placement:
  sync (SP):     all HBM DMAs (x/W in, y out)
  vector (DVE):  x casts
  scalar (Act):  W casts + all XBAR issues (each xbar waits only its cast)
  any:           psum->sbuf y evictions (scheduler balances Act/DVE)
"""
import numpy as np

import concourse.bass as bass
import concourse.mybir as mybir
import concourse.tile as tile
from concourse import bacc
from concourse.bass_utils import run_bass_kernel_spmd

F32 = mybir.dt.float32
BF16 = mybir.dt.bfloat16
P = 128

N_CORES = 8
B = 4096
K = 4096
O = 4096
BGRID = 4            # batch shards
OGRID = 2            # out shards
B_S = B // BGRID     # 1024 batch rows per core
O_S = O // OGRID     # 2048 out cols per core


def build(B_S=B_S, K=K, O_S=O_S, OG=512, n_cores=N_CORES):
    KT = K // P           # 32 k-tiles
    BT = B_S // P         # 8 b-tiles (one psum bank each)
    NOG = O_S // OG       # 4 o-groups
    OT = OG // P          # 4 o-row-blocks per o-group

    nc = bacc.Bacc("TRN2", target_bir_lowering=False, debug=False,
                   num_devices=n_cores)
    x = nc.dram_tensor("x", [B_S, K], F32, kind="ExternalInput").ap()
    w = nc.dram_tensor("w", [O_S, K], F32, kind="ExternalInput").ap()
    b = nc.dram_tensor("b", [O_S], F32, kind="ExternalInput").ap()
    y = nc.dram_tensor("y", [B_S, O_S], F32, kind="ExternalOutput").ap()

    with tile.TileContext(nc) as tc:
        with tc.tile_pool(name="const", bufs=1) as const, \
             tc.tile_pool(name="nat", bufs=2) as nat_pool, \
             tc.tile_pool(name="bf", bufs=2) as bf_pool, \
             tc.tile_pool(name="xt", bufs=1) as xt_pool, \
             tc.tile_pool(name="wt", bufs=2) as wt_pool, \
             tc.tile_pool(name="yo", bufs=3) as yo_pool, \
             tc.tile_pool(name="yps", bufs=1, space="PSUM") as yps:

            bias_f = const.tile([1, O_S], F32)
            nc.sync.dma_start(bias_f, b.unsqueeze(0))
            bias_sb = const.tile([1, O_S], BF16)
            nc.vector.tensor_copy(bias_sb, bias_f)
            ones_f = const.tile([1, P], F32)
            nc.any.memset(ones_f, 1.0)
            ones_k1 = const.tile([1, P], BF16)
            nc.vector.tensor_copy(ones_k1, ones_f)

            xT = xt_pool.tile([P, KT, B_S], BF16)   # xT[p, kt, b] = x[b, kt*P+p]

            # -- x ingest: dma(sync) -> cast(DVE) -> xbar(Act queue) --
            def x_ingest(bt):
                x_nat = nat_pool.tile([P, K], F32, tag="nat")
                nc.sync.dma_start(x_nat, x[bt * P:(bt + 1) * P, :])
                x_bf = bf_pool.tile([P, K], BF16, tag="bf")
                nc.vector.tensor_copy(x_bf, x_nat)
                nc.scalar.dma_start(
                    xT[:, :, bt * P:(bt + 1) * P], x_bf, transpose=True)

            # -- W ingest for one o-row-block: dma(sync) -> cast(Act) -> xbar(Act) --
            def w_ingest(wT_cur, og, ot):
                r0 = og * OG + ot * P
                w_nat = nat_pool.tile([P, K], F32, tag="nat")
                nc.sync.dma_start(w_nat, w[r0:r0 + P, :])
                w_bf = bf_pool.tile([P, K], BF16, tag="bf")
                nc.scalar.copy(w_bf, w_nat)
                nc.scalar.dma_start(
                    wT_cur[:, :, ot * P:(ot + 1) * P], w_bf, transpose=True)

            wT = [None] * NOG
            # Prime the pipeline: W o-group 0 first (it gates all og-0
            # matmuls), then the x b-tiles in use order.
            wT[0] = wt_pool.tile([P, KT, OG], BF16, name="wT0", tag="wT")
            for ot in range(OT):
                w_ingest(wT[0], 0, ot)
            for bt in range(BT):
                x_ingest(bt)

            for og in range(NOG):
                if og + 1 < NOG:
                    wT[og + 1] = wt_pool.tile([P, KT, OG], BF16,
                                              name=f"wT{og + 1}", tag="wT")
                    for ot in range(OT):
                        w_ingest(wT[og + 1], og + 1, ot)

                for bt in range(BT):
                    psum_y = yps.tile([P, OG], F32, name=f"psum_y{og}_{bt}",
                                      tag=f"psum_y{bt}")
                    nc.tensor.matmul(
                        psum_y, ones_k1,
                        bias_sb[:, og * OG:(og + 1) * OG],
                        start=True, stop=False)
                    for k in range(KT):
                        nc.tensor.matmul(
                            psum_y,
                            xT[:, k, bt * P:(bt + 1) * P],
                            wT[og][:, k, :],
                            start=False,
                            stop=(k == KT - 1),
                        )
                    y_sb = yo_pool.tile([P, OG], F32, tag="y_sb")
                    nc.any.tensor_copy(y_sb, psum_y)
                    nc.sync.dma_start(
                        y[bt * P:(bt + 1) * P, og * OG:(og + 1) * OG], y_sb)
                wT[og] = None

    nc.compile()
    return nc


_nc_cache = {}


def get_nc():
    if "nc" not in _nc_cache:
        _nc_cache["nc"] = build()
    return _nc_cache["nc"]


def make_in_maps(x, weight, bias):
    x = np.ascontiguousarray(np.asarray(x, dtype=np.float32))
    weight = np.ascontiguousarray(np.asarray(weight, dtype=np.float32))
    bias = np.ascontiguousarray(np.asarray(bias, dtype=np.float32))
    assert x.shape == (B, K) and weight.shape == (O, K) and bias.shape == (O,)
    maps = []
    for c in range(N_CORES):
        bi, oi = c // OGRID, c % OGRID
        maps.append({
            "x": np.ascontiguousarray(x[bi * B_S:(bi + 1) * B_S]),
            "w": np.ascontiguousarray(weight[oi * O_S:(oi + 1) * O_S]),
            "b": np.ascontiguousarray(bias[oi * O_S:(oi + 1) * O_S]),
        })
    return maps


def run(x, weight, bias, **spmd_kwargs):
    """Run on all 8 cores; returns (y_full, BassKernelResults)."""
    nc = get_nc()
    in_maps = make_in_maps(x, weight, bias)
    res = run_bass_kernel_spmd(nc, in_maps, list(range(N_CORES)), **spmd_kwargs)
    y_full = np.empty((B, O), dtype=np.float32)
    for c in range(N_CORES):
        bi, oi = c // OGRID, c % OGRID
        y_full[bi * B_S:(bi + 1) * B_S, oi * O_S:(oi + 1) * O_S] = \
            res.results[c]["y"]
    return y_full, res


def kernel(x, weight, bias):
    y, _ = run(x, weight, bias)
    return y
